# revision 1
# baseline (speedup 1.0000x reference)
"""Paged attention (decode) on 8 Trainium2 NeuronCores.

Sharding: tensor-parallel over KV heads — core h owns kv head h, its 4
query heads, and the per-head slices of both caches.

Per-core algorithm (all layouts chosen so softmax runs with tokens on
partitions and no on-chip transposes are needed):

  host prep:
    K cache head-slice  -> 4-token rows [t'(4) x d(128)], split into
                           bf16 hi/lo pair (hi+lo == fp32 value to ~2^-17)
    V cache head-slice  -> 4-token rows [t'(4) x d(128)] fp32
    Q head-slice        -> QT [d, seq, 8] bf16: cols 0:4 = hi, 4:8 = lo
    block tables        -> per-seq row-index lists (int16), padded to 128
    masks               -> 0/1 validity for each seq's last 512-token group

  device, per sequence:
    dma_gather(transpose=True)  on K rows -> KT tiles [d=128, t', idx]
    dma_gather(transpose=False) on V rows -> V tiles [t_grp=128, slot, t'*d]
    QK:  S^T chunk [t=128, 8] = (Kh|Kl slab).T @ [Qh|Ql]  (2 matmuls, PSUM)
    fold hi/lo halves + exp(scale*S) + multiplicative mask -> P [t, ct, q]
    PV:  out^T [d=128, q] += V_slab.T? no: lhsT=V slab [t,d], rhs=P slab [t,q]
         accumulated over chunks into a per-seq PSUM column block
    denom: ones[t,1].T @ P -> [1, ct*q] -> reduced to [1, q]
  epilogue: replicate denominators across partitions with a 1-partition
  matmul, reciprocal, scale, DMA out^T [d, seq*4+q] to DRAM.
"""

import numpy as np
import ml_dtypes

import concourse.bass as bass
import concourse.mybir as mybir
import concourse.tile as tile
from concourse import bacc
from concourse.bass_utils import run_bass_kernel_spmd

BF16 = ml_dtypes.bfloat16
BLOCK_SIZE = 16
ROWS_PER_BLOCK = 4          # 4-token rows
TOK_PER_ROW = 4


def _build_program(n_blocks, n_seqs, n_list, offs, nslot, repeat=1):
    """Build the (single, SPMD) Bass program.

    n_list[s]: padded row count for seq s (multiple of 128)
    offs[s]:   int16-slot offset of seq s's indices (units of 16 idxs)
    nslot:     total idx slots (free dim of the idx tensor)
    """
    D = 128
    nrows_total = n_blocks * ROWS_PER_BLOCK
    f32 = mybir.dt.float32
    bf16 = mybir.dt.bfloat16

    nc = bacc.Bacc("TRN2", target_bir_lowering=False, debug=False)
    kh_d = nc.dram_tensor("krows_hi", [nrows_total, 512], bf16, kind="ExternalInput")
    kl_d = nc.dram_tensor("krows_lo", [nrows_total, 512], bf16, kind="ExternalInput")
    v_d = nc.dram_tensor("vrows", [nrows_total, 512], f32, kind="ExternalInput")
    q_d = nc.dram_tensor("qthl", [D, n_seqs, 8], bf16, kind="ExternalInput")
    idx_d = nc.dram_tensor("idx", [128, nslot], mybir.dt.int16, kind="ExternalInput")
    m_d = nc.dram_tensor("masks", [128, n_seqs, 4, 4], f32, kind="ExternalInput")
    o_d = nc.dram_tensor("out", [D, n_seqs * 4], f32, kind="ExternalOutput")

    with tile.TileContext(nc) as tc:
        with (
            tc.tile_pool(name="const", bufs=1) as const,
            tc.tile_pool(name="kh", bufs=4) as khp,
            tc.tile_pool(name="kl", bufs=4) as klp,
            tc.tile_pool(name="vp", bufs=4) as vp,
            tc.tile_pool(name="pp", bufs=2) as pp,
            tc.tile_pool(name="psS", bufs=2, space="PSUM") as psSp,
            tc.tile_pool(name="psO", bufs=1, space="PSUM") as psOp,
            tc.tile_pool(name="psD", bufs=2, space="PSUM") as psDp,
            tc.tile_pool(name="psR", bufs=1, space="PSUM") as psRp,
        ):
            qthl = const.tile([D, n_seqs, 8], bf16)
            idx_t = const.tile([128, nslot], mybir.dt.int16)
            masks = const.tile([128, n_seqs, 4, 4], f32)
            ones = const.tile([128, 1], f32)
            onesr = const.tile([1, 128], f32)
            dsb = const.tile([1, n_seqs, 4], f32)
            outsb = const.tile([D, n_seqs * 4], f32)
            recs = const.tile([D, n_seqs * 4], f32)
            nc.sync.dma_start(qthl[:], q_d[:])
            nc.sync.dma_start(idx_t[:], idx_d[:])
            nc.sync.dma_start(masks[:], m_d[:])
            nc.vector.memset(ones[:], 1.0)
            nc.vector.memset(onesr[:], 1.0)

            psO = psOp.tile([D, n_seqs * 4], f32)

            scale = float(1.0 / np.sqrt(np.float32(D)).astype(np.float32))

            GCH = 6  # c-groups (of 128 rows) per gather instruction; 768
            #          rows per gather is under the ~832-row HW gather limit
            for s in [s for _ in range(repeat) for s in range(n_seqs)]:
                n = n_list[s]
                C = n // 128
                kh_ts, kl_ts, v_ts = [], [], []
                for g in range(0, C, GCH):
                    ln = min(GCH, C - g) * 128
                    kh_t = khp.tile([D, 4, ln], bf16, tag="kh")
                    kl_t = klp.tile([D, 4, ln], bf16, tag="kl")
                    v_t = vp.tile([128, ln // 128, 512], f32, tag="v")
                    i_ap = idx_t[:, offs[s] + g * 8:offs[s] + g * 8 + ln // 16]
                    nc.gpsimd.dma_gather(kh_t[:], kh_d[:], i_ap, ln, ln, 512,
                                         transpose=True)
                    nc.gpsimd.dma_gather(kl_t[:], kl_d[:], i_ap, ln, ln, 512,
                                         transpose=True)
                    nc.gpsimd.dma_gather(v_t[:], v_d[:], i_ap, ln, ln, 512)
                    kh_ts.append(kh_t)
                    kl_ts.append(kl_t)
                    v_ts.append(v_t)

                psS = psSp.tile([128, C * 4, 4], f32, tag="psS")
                rhs_qh = qthl[:, s, 0:4]
                rhs_ql = qthl[:, s, 4:8]
                for c in range(C):
                    g, cl = c // GCH, c % GCH
                    for tp in range(4):
                        out_ap = psS[:, c * 4 + tp, :]
                        kh_slab = kh_ts[g][:, tp, cl * 128:(cl + 1) * 128]
                        kl_slab = kl_ts[g][:, tp, cl * 128:(cl + 1) * 128]
                        nc.tensor.matmul(out_ap, kh_slab, rhs_qh,
                                         start=True, stop=False)
                        nc.tensor.matmul(out_ap, kh_slab, rhs_ql,
                                         start=False, stop=False)
                        nc.tensor.matmul(out_ap, kl_slab, rhs_qh,
                                         start=False, stop=True)

                p_t = pp.tile([128, C * 4, 4], f32, tag="p")
                nc.scalar.activation(p_t[:], psS[:],
                                     mybir.ActivationFunctionType.Exp,
                                     scale=scale)
                nc.vector.tensor_mul(p_t[:, (C - 1) * 4:C * 4, :],
                                     p_t[:, (C - 1) * 4:C * 4, :],
                                     masks[:, s, :, :])

                for c in range(C):
                    g, cl = c // GCH, c % GCH
                    for tp in range(4):
                        nc.tensor.matmul(psO[:, s * 4:(s + 1) * 4],
                                         v_ts[g][:, cl, tp * 128:(tp + 1) * 128],
                                         p_t[:, c * 4 + tp, :],
                                         start=(c == 0 and tp == 0),
                                         stop=(c == C - 1 and tp == 3))

                psD = psDp.tile([1, 4, C * 4], f32, tag="psD")
                nc.tensor.matmul(psD[:], ones[:],
                                 p_t[:].transpose([0, 2, 1]),
                                 start=True, stop=True)
                nc.vector.tensor_reduce(dsb[:, s, :], psD[:],
                                        mybir.AxisListType.X,
                                        mybir.AluOpType.add)

            # epilogue: replicate denominators to all partitions, divide
            psR = psRp.tile([128, n_seqs * 4], f32)
            nc.tensor.matmul(psR[:], onesr[:], dsb[0:1, :, :],
                             start=True, stop=True)
            nc.vector.reciprocal(recs[:], psR[:])
            nc.vector.tensor_mul(outsb[:], psO[:], recs[:])
            nc.sync.dma_start(o_d[:], outsb[:])

    nc.compile()
    return nc


def prepare(query, key_cache, value_cache, block_tables, context_lens,
            repeat=1):
    """Host prep + program build. Returns (nc, in_maps, (B, H, D, kvh))."""
    query = np.asarray(query)
    key_cache = np.asarray(key_cache)
    value_cache = np.asarray(value_cache)
    block_tables = np.asarray(block_tables)
    context_lens = np.asarray(context_lens)

    nb_tot, kvh, dx, bs, x = key_cache.shape
    D = dx * x
    B, H, _ = query.shape
    qpk = H // kvh
    assert D == 128 and bs == BLOCK_SIZE and qpk == 4 and B * 4 <= 128

    ctx = context_lens.astype(np.int64)
    nb = -(-ctx // BLOCK_SIZE)                       # blocks per seq
    nrows = ROWS_PER_BLOCK * nb                      # 4-token rows per seq
    n_list = (-(-nrows // 128) * 128).astype(np.int64)  # padded to 128
    C_list = n_list // 128
    offs = np.zeros(B, dtype=np.int64)
    acc = 0
    for s in range(B):
        offs[s] = acc
        acc += n_list[s] // 16
    nslot = int(acc)

    # ---- shared (head-independent) host prep ----
    # per-seq index lists
    idx_flat = np.zeros(nslot * 16, dtype=np.int16)
    for s in range(B):
        m = int(nrows[s])
        rows = (block_tables[s, :nb[s], None] * ROWS_PER_BLOCK
                + np.arange(ROWS_PER_BLOCK)[None, :]).reshape(-1).astype(np.int16)
        base = offs[s] * 16
        idx_flat[base:base + m] = rows
    idx_wrapped = np.ascontiguousarray(
        idx_flat.reshape(nslot, 16).T)               # [16, nslot]
    idx_rep = np.ascontiguousarray(np.tile(idx_wrapped, (8, 1)))  # [128, nslot]

    # masks [128, B, 4, 4]: validity of each seq's LAST 512-token group
    p_ar = np.arange(128)
    tp_ar = np.arange(4)
    L = 16 * (p_ar[:, None] // 4) + 4 * (p_ar[:, None] % 4) + tp_ar[None, :]
    masks = np.zeros((128, B, 4, 4), dtype=np.float32)
    for s in range(B):
        rem = int(ctx[s]) - 512 * (int(C_list[s]) - 1)
        masks[:, s, :, 0] = (L < rem).astype(np.float32)
    masks[:, :, :, 1:] = masks[:, :, :, 0:1]

    # ---- per-core prep ----
    in_maps = []
    for h in range(kvh):
        kc = key_cache[:, h]                          # [NB, dx, bs, x]
        K = np.ascontiguousarray(kc.transpose(0, 2, 1, 3)).reshape(nb_tot, bs, D)
        Krows = K.reshape(nb_tot * ROWS_PER_BLOCK, TOK_PER_ROW * D)
        khi = Krows.astype(BF16)
        klo = (Krows - khi.astype(np.float32)).astype(BF16)

        vc = value_cache[:, h]                        # [NB, D, bs]
        V = np.ascontiguousarray(vc.transpose(0, 2, 1))  # [NB, bs, D]
        vrows = V.reshape(nb_tot * ROWS_PER_BLOCK, TOK_PER_ROW * D)

        qh = query[:, 4 * h:4 * h + 4, :]             # [B, 4, D]
        qt = np.ascontiguousarray(qh.transpose(2, 0, 1))  # [D, B, 4]
        qhi = qt.astype(BF16)
        qlo = (qt - qhi.astype(np.float32)).astype(BF16)
        qthl = np.concatenate([qhi, qlo], axis=2)     # [D, B, 8]

        in_maps.append({
            "krows_hi": khi, "krows_lo": klo, "vrows": vrows,
            "qthl": np.ascontiguousarray(qthl), "idx": idx_rep,
            "masks": masks,
        })

    build_args = (nb_tot, B, [int(v) for v in n_list],
                  [int(v) for v in offs], nslot)
    globals()["_last_build_args"] = build_args
    nc = _build_program(*build_args, repeat=repeat)
    return nc, in_maps, (B, H, D, kvh)


def assemble(res, meta):
    B, H, D, kvh = meta
    out = np.empty((B, H, D), dtype=np.float32)
    for h in range(kvh):
        o = res[h]["out"]                             # [D, B*4]
        out[:, 4 * h:4 * h + 4, :] = o.reshape(D, B, 4).transpose(1, 2, 0)
    return out


def kernel(query, key_cache, value_cache, block_tables, context_lens):
    nc, in_maps, meta = prepare(query, key_cache, value_cache,
                                block_tables, context_lens)
    kres = run_bass_kernel_spmd(nc, in_maps, list(range(meta[3])))
    globals()["_last_results"] = kres
    return assemble(kres.results, meta)



# revision 3
# speedup vs baseline: 671.3068x; 671.3068x over previous
"""Paged attention (decode) on 8 Trainium2 NeuronCores — block-granularity
gathers.

Sharding: tensor-parallel over KV heads — core h owns kv head h, its 4
query heads, and the per-head slices of both caches.

vs the 4-token-row version: gathers move whole 16-token blocks (4 KB
descriptors instead of 1 KB, 4.4x fewer descriptors) and only the blocks
each sequence actually references (no padding to 512-token groups).

Layouts:
  K blocks -> krows [NB, 2048] bf16, row b = K[b] as [tok, d] flattened.
     transpose-gathered (bin-packed across seqs, <=768 idxs per gather)
     into KT tiles [d=128, j(16), pos]: KT[d, j, i] = K[blk_i][j*128+d].
  V blocks -> vrows [NB, 2048] bf16, same row layout.
     plain-gathered per seq into V tiles [pos%128, pos//128, 2048].
  chunk = up to 128 consecutive blocks of one seq (<=2048 tokens).
  QK per (chunk, j):  psS[0:mc, c*16+j, q] = KT_slab[d, mc].T @ Q[d, 4]
  exp per chunk (partial partitions: stale PSUM is never read)
  PV per (chunk, j):  psO[:, s*4+q] += V_slab[0:mc, d] . P[0:mc, 4]
  denominators: ones.T @ P in two matmuls (full chunks + partial last)
"""

import numpy as np
import ml_dtypes

import concourse.bass as bass
import concourse.mybir as mybir
import concourse.tile as tile
from concourse import bacc
from concourse.bass_utils import run_bass_kernel_spmd

BF16 = ml_dtypes.bfloat16
BLOCK_SIZE = 16
BIN_CAP = 768               # blocks per K gather (HW gather limit ~832)


def _plan(ctx, B):
    """Bin-pack seqs' block lists into K gathers; lay out idx streams.

    Returns dict with per-seq (nb, C, m, bin id, a=offset in bin) and
    per-bin (klen padded to 128); plus V idx slot offsets (16-aligned).
    """
    nb = [-(-int(c) // BLOCK_SIZE) for c in ctx]
    bins = []                # list of [seq ids]
    fill = []                # current block count per bin
    binof = [0] * B
    aoff = [0] * B
    for s in range(B):
        placed = False
        for g in range(len(bins)):
            if fill[g] + nb[s] <= BIN_CAP:
                binof[s], aoff[s] = g, fill[g]
                bins[g].append(s)
                fill[g] += nb[s]
                placed = True
                break
        if not placed:
            binof[s], aoff[s] = len(bins), 0
            bins.append([s])
            fill.append(nb[s])
    klen = [-(-f // 128) * 128 for f in fill]
    kboff = np.cumsum([0] + [k // 16 for k in klen])[:-1]  # idx slot offsets
    voff = np.cumsum([0] + [-(-n // 16) for n in nb])[:-1]
    return {
        "nb": nb, "bins": bins, "klen": klen,
        "kboff": [int(x) for x in kboff], "aoff": aoff, "binof": binof,
        "voff": [int(x) for x in voff],
        "nkslot": int(sum(k // 16 for k in klen)),
        "nvslot": int(sum(-(-n // 16) for n in nb)),
    }


def _build_program(n_blocks, n_seqs, plan, repeat=1):
    D = 128
    f32 = mybir.dt.float32
    bf16 = mybir.dt.bfloat16
    nb, bins, klen = plan["nb"], plan["bins"], plan["klen"]
    kboff, aoff, voff = plan["kboff"], plan["aoff"], plan["voff"]
    nkslot, nvslot = plan["nkslot"], plan["nvslot"]

    nc = bacc.Bacc("TRN2", target_bir_lowering=False, debug=False)
    k_d = nc.dram_tensor("krows", [n_blocks, 2048], bf16, kind="ExternalInput")
    v_d = nc.dram_tensor("vrows", [n_blocks, 2048], bf16, kind="ExternalInput")
    q_d = nc.dram_tensor("qt", [D, n_seqs, 4], bf16, kind="ExternalInput")
    ik_d = nc.dram_tensor("idxk", [128, nkslot], mybir.dt.int16,
                          kind="ExternalInput")
    iv_d = nc.dram_tensor("idxv", [128, nvslot], mybir.dt.int16,
                          kind="ExternalInput")
    m_d = nc.dram_tensor("masks", [128, n_seqs, 16, 4], bf16,
                         kind="ExternalInput")
    o_d = nc.dram_tensor("out", [D, n_seqs * 4], f32, kind="ExternalOutput")

    with tile.TileContext(nc) as tc:
        with (
            tc.tile_pool(name="const", bufs=1) as const,
            tc.tile_pool(name="kp", bufs=2) as kp,
            tc.tile_pool(name="vp", bufs=4) as vp,
            tc.tile_pool(name="pp", bufs=2) as pp,
            tc.tile_pool(name="psS", bufs=2, space="PSUM") as psSp,
            tc.tile_pool(name="psO", bufs=1, space="PSUM") as psOp,
            tc.tile_pool(name="psD", bufs=2, space="PSUM") as psDp,
            tc.tile_pool(name="psR", bufs=1, space="PSUM") as psRp,
        ):
            qt = const.tile([D, n_seqs, 4], bf16)
            idxk = const.tile([128, nkslot], mybir.dt.int16)
            idxv = const.tile([128, nvslot], mybir.dt.int16)
            masks = const.tile([128, n_seqs, 16, 4], bf16)
            ones = const.tile([128, 1], bf16)
            onesr = const.tile([1, 128], f32)
            dsb = const.tile([1, n_seqs, 4], f32)
            outsb = const.tile([D, n_seqs * 4], f32)
            recs = const.tile([D, n_seqs * 4], f32)
            nc.sync.dma_start(qt[:], q_d[:])
            nc.sync.dma_start(idxk[:], ik_d[:])
            nc.sync.dma_start(idxv[:], iv_d[:])
            nc.sync.dma_start(masks[:], m_d[:])
            nc.vector.memset(ones[:], 1.0)
            nc.vector.memset(onesr[:], 1.0)

            psO = psOp.tile([D, n_seqs * 4], f32)
            scale = float(1.0 / np.sqrt(np.float32(D)).astype(np.float32))

            for _ in range(repeat):
                for g, bin_seqs in enumerate(bins):
                    L = klen[g]
                    k_t = kp.tile([D, 16, L], bf16, tag="k")
                    nc.gpsimd.dma_gather(
                        k_t[:], k_d[:],
                        idxk[:, kboff[g]:kboff[g] + L // 16],
                        L, L, 2048, transpose=True)

                    for s in bin_seqs:
                        n = nb[s]
                        C = -(-n // 128)
                        m = n - (C - 1) * 128
                        a = aoff[s]
                        v_t = vp.tile([128, C, 2048], bf16, tag="v")
                        nc.gpsimd.dma_gather(
                            v_t[:], v_d[:],
                            idxv[:, voff[s]:voff[s] + (-(-n // 16))],
                            n, n, 2048)

                        psS = psSp.tile([128, C * 16, 4], f32, tag="psS")
                        rhs_q = qt[:, s, :]
                        for c in range(C):
                            mc = 128 if c < C - 1 else m
                            base = a + c * 128
                            for j in range(16):
                                nc.tensor.matmul(
                                    psS[0:mc, c * 16 + j, :],
                                    k_t[:, j, base:base + mc],
                                    rhs_q, start=True, stop=True)

                        p_t = pp.tile([128, C * 16, 4], bf16, tag="p")
                        for c in range(C):
                            mc = 128 if c < C - 1 else m
                            nc.scalar.activation(
                                p_t[0:mc, c * 16:(c + 1) * 16, :],
                                psS[0:mc, c * 16:(c + 1) * 16, :],
                                mybir.ActivationFunctionType.Exp,
                                scale=scale)
                        nc.vector.tensor_mul(
                            p_t[0:m, (C - 1) * 16:C * 16, :],
                            p_t[0:m, (C - 1) * 16:C * 16, :],
                            masks[0:m, s, :, :])

                        for c in range(C):
                            mc = 128 if c < C - 1 else m
                            for j in range(16):
                                nc.tensor.matmul(
                                    psO[:, s * 4:(s + 1) * 4],
                                    v_t[0:mc, c, j * 128:(j + 1) * 128],
                                    p_t[0:mc, c * 16 + j, :],
                                    start=(c == 0 and j == 0),
                                    stop=(c == C - 1 and j == 15))

                        psD = psDp.tile([1, 4, C * 16], f32, tag="psD")
                        if C > 1:
                            nc.tensor.matmul(
                                psD[:, :, 0:(C - 1) * 16], ones[:],
                                p_t[:, 0:(C - 1) * 16, :].transpose([0, 2, 1]),
                                start=True, stop=True)
                        nc.tensor.matmul(
                            psD[:, :, (C - 1) * 16:C * 16], ones[0:m, :],
                            p_t[0:m, (C - 1) * 16:C * 16, :].transpose([0, 2, 1]),
                            start=True, stop=True)
                        nc.vector.tensor_reduce(dsb[:, s, :], psD[:],
                                                mybir.AxisListType.X,
                                                mybir.AluOpType.add)

            # epilogue: replicate denominators to all partitions, divide
            psR = psRp.tile([128, n_seqs * 4], f32)
            nc.tensor.matmul(psR[:], onesr[:], dsb[0:1, :, :],
                             start=True, stop=True)
            nc.vector.reciprocal(recs[:], psR[:])
            nc.vector.tensor_mul(outsb[:], psO[:], recs[:])
            nc.sync.dma_start(o_d[:], outsb[:])

    nc.compile()
    return nc


def _wrap_idx(flat_positions, nslot, values):
    """Pack an int16 idx list into the [128, nslot] wrapped+replicated
    layout: position i -> (row i%16, slot i//16), tiled 8x to 128 rows."""
    idx_flat = np.zeros(nslot * 16, dtype=np.int16)
    idx_flat[:len(values)] = values
    wrapped = np.ascontiguousarray(idx_flat.reshape(nslot, 16).T)
    return np.ascontiguousarray(np.tile(wrapped, (8, 1)))


def prepare(query, key_cache, value_cache, block_tables, context_lens,
            repeat=1):
    query = np.asarray(query)
    key_cache = np.asarray(key_cache)
    value_cache = np.asarray(value_cache)
    block_tables = np.asarray(block_tables)
    context_lens = np.asarray(context_lens)

    nb_tot, kvh, dx, bs, x = key_cache.shape
    D = dx * x
    B, H, _ = query.shape
    qpk = H // kvh
    assert D == 128 and bs == BLOCK_SIZE and qpk == 4 and B * 4 <= 128

    ctx = context_lens.astype(np.int64)
    plan = _plan(ctx, B)
    nb = plan["nb"]

    # K idx stream (bin-packed), V idx stream (per-seq, 16-aligned)
    kvals = np.zeros(plan["nkslot"] * 16, dtype=np.int16)
    for g, bin_seqs in enumerate(plan["bins"]):
        base = plan["kboff"][g] * 16
        o = base
        for s in bin_seqs:
            kvals[o:o + nb[s]] = block_tables[s, :nb[s]].astype(np.int16)
            o += nb[s]
    idxk = _wrap_idx([], plan["nkslot"], kvals)  # kvals already full-size
    idxk_flat = kvals
    idxk = np.ascontiguousarray(np.tile(np.ascontiguousarray(
        idxk_flat.reshape(plan["nkslot"], 16).T), (8, 1)))

    vvals = np.zeros(plan["nvslot"] * 16, dtype=np.int16)
    for s in range(B):
        base = plan["voff"][s] * 16
        vvals[base:base + nb[s]] = block_tables[s, :nb[s]].astype(np.int16)
    idxv = np.ascontiguousarray(np.tile(np.ascontiguousarray(
        vvals.reshape(plan["nvslot"], 16).T), (8, 1)))

    # masks [128, B, 16, 4]: validity of the LAST chunk of each seq.
    # partition p = block (C-1)*128+p of the seq, col (j, q):
    # valid iff ((C-1)*128+p)*16 + j < ctx
    p_ar = np.arange(128)[:, None]
    j_ar = np.arange(16)[None, :]
    masks = np.zeros((128, B, 16, 4), dtype=np.float32)
    for s in range(B):
        C = -(-nb[s] // 128)
        tok = ((C - 1) * 128 + p_ar) * 16 + j_ar     # [128, 16]
        masks[:, s, :, 0] = (tok < int(ctx[s])).astype(np.float32)
    masks[:, :, :, 1:] = masks[:, :, :, 0:1]
    masks = masks.astype(BF16)

    in_maps = []
    for h in range(kvh):
        kc = key_cache[:, h]                          # [NB, dx, bs, x]
        K = np.ascontiguousarray(kc.transpose(0, 2, 1, 3)).reshape(nb_tot, -1)
        krows = K.astype(BF16)                        # [NB, 2048] tok-major

        vc = value_cache[:, h]                        # [NB, D, bs]
        V = np.ascontiguousarray(vc.transpose(0, 2, 1)).reshape(nb_tot, -1)
        vrows = V.astype(BF16)                        # [NB, 2048] tok-major

        qh = query[:, 4 * h:4 * h + 4, :]             # [B, 4, D]
        qt = np.ascontiguousarray(
            qh.transpose(2, 0, 1)).astype(BF16)       # [D, B, 4]

        in_maps.append({
            "krows": krows, "vrows": vrows, "qt": qt,
            "idxk": idxk, "idxv": idxv, "masks": masks,
        })

    build_args = (nb_tot, B, plan)
    globals()["_last_build_args"] = build_args
    nc = _build_program(*build_args, repeat=repeat)
    return nc, in_maps, (B, H, D, kvh)


def assemble(res, meta):
    B, H, D, kvh = meta
    out = np.empty((B, H, D), dtype=np.float32)
    for h in range(kvh):
        o = res[h]["out"]                             # [D, B*4]
        out[:, 4 * h:4 * h + 4, :] = o.reshape(D, B, 4).transpose(1, 2, 0)
    return out


def kernel(query, key_cache, value_cache, block_tables, context_lens):
    nc, in_maps, meta = prepare(query, key_cache, value_cache,
                                block_tables, context_lens)
    kres = run_bass_kernel_spmd(nc, in_maps, list(range(meta[3])))
    globals()["_last_results"] = kres
    return assemble(kres.results, meta)
